# revision 46
# baseline (speedup 1.0000x reference)
"""Trainium2 Bass kernel for nn_AttnApproximator (GQA attention + RoPE +
per-head shift correction), sharded over 8 NeuronCores.

Sharding: tensor-parallel over heads (4 groups of 8 query heads / 2 KV
heads) x data-parallel over batch (B=2) -> 8 cores. Each core computes a
partial output contribution [S, Dm] (its heads' slice of the attn @ Wo
contraction); the host sums the 4 head-group partials per batch element.

Fully fused per-slice pipeline (one TileContext scope, the Tile list
scheduler interleaves streams by readiness):
  slice n: kv-proj(n) -> q-proj(n) -> attention(n); after odd slices an
  o-proj pass covers the previous two slices. Attention's serial
  score->exp->attnV chains leave PE gaps that the scheduler fills with
  projection / o-proj matmuls emitted around them.

vs the phase-split baseline (894us):
  - softmax denominators: exp tiles are accumulated on DVE (bf16 adds)
    and reduced with ONE ones[128x128] matmul per (head, slice) which
    also broadcasts the sums to all partitions -- replaces per-k-tile
    ones matmuls AND the separate broadcast matmul (~75us of PE).
  - RoPE runs in bf16 (DVE 2x mode) off a single ACT drain.
  - hsT is loaded once (fused slices), o-proj reads Wo twice total.
"""

import math
import numpy as np
import ml_dtypes

import bass_rust
import concourse.bass as bass
import concourse.tile as tile
from concourse import mybir
from concourse import bass_utils
from concourse.vector_clock import ScopedClock
from contextlib import ExitStack

# ---------------------------------------------------------------- constants
B, S, DM = 2, 2048, 4096
H, KV, D = 32, 8, 128
N_CORES = 8
TP = 4                    # head groups
HQ = H // TP              # 8 q heads per core
HKV = KV // TP            # 2 kv heads per core
GROUPS = H // KV          # 4
THETA = 10000.0
SQ = 512                  # s-slice width
NSL = S // SQ             # 4 slices
NK = DM // 128            # 32 contraction tiles
SCALE = 1.0 / math.sqrt(D)

F32 = mybir.dt.float32
BF16 = mybir.dt.bfloat16
BF_NP = ml_dtypes.bfloat16


# ------------------------------------------------- walrus drain-wait fixup
class SplitDrainTileContext(tile.TileContext):
    """This container's walrus rejects >1 sync wait on the SP tail-drain
    CTRL instruction; split the gathered waits onto chained SP nops."""

    MAX_WAITS = 1

    def _drain_and_barrier(self, tick_clock, wait_clock):
        nc = self.nc
        drain_inst = nc.sync.drain()
        wait_clock.add_sem_waits(
            drain_inst.ins, ScopedClock({None: tick_clock.global_clock})
        )
        si = drain_inst.ins.sync_info
        waits = list(si.on_wait) if si is not None else []
        mw = self.MAX_WAITS
        if len(waits) > mw:
            drain_inst.ins.sync_info = bass_rust.SyncInfo(
                on_wait=waits[:mw], on_update=list(si.on_update)
            )
            for k in range(mw, len(waits), mw):
                nop = nc.sync.nop(nofuse=True, hint="drain_wait_split")
                nop.ins.sync_info = bass_rust.SyncInfo(
                    on_wait=waits[k : k + mw], on_update=[]
                )
        nc.all_engine_barrier()
        assert self.sems is not None
        popped = nc._tile_sem_poison_stack.pop()
        assert popped is self._sem_poison
        nc.clear_and_free_semaphores(list(self.sems.allocated().values()))
        nc.all_engine_barrier()


def _split_excess_waits(nc):
    """This walrus accepts 1 sync wait per instruction (2 for
    EventSemaphore). Tile emits more; move the excess onto same-engine
    NoOp carriers inserted immediately before the over-limit instruction."""
    uid = 0
    for fn in nc.m.functions:
        for bb in fn.blocks:
            new, changed = [], False
            for inst in bb.instructions:
                si = inst.sync_info
                waits = list(si.on_wait) if si is not None else []
                cap = 2 if inst.opcode == "EventSemaphore" else 1
                if len(waits) > cap:
                    changed = True
                    for w in waits[:-cap]:
                        nop = mybir.InstNoOp(
                            name=f"I-wsplit-{uid}",
                            engine=inst.engine,
                            bass_nofuse=True,
                            sync_info=mybir.SyncInfo(on_wait=[w], on_update=[]),
                        )
                        uid += 1
                        new.append(nop)
                    inst.sync_info = bass_rust.SyncInfo(
                        on_wait=waits[-cap:], on_update=list(si.on_update))
                new.append(inst)
            if changed:
                bb.instructions = new


# ---------------------------------------------------------------- builder
def _rope(nc, tmp_pool, out_ap, in_ps, cos_sl, sin_sl):
    """out = in*cosT + swap_halves(in)*sinT_signed ; in_ps is PSUM f32.
    ACT drains PSUM with a cast to bf16 so the DVE muls/adds run in the
    2x packed mode; the bank frees fast for the next projection chain."""
    q_sb = tmp_pool.tile([128, SQ], BF16, tag="rope_q")
    nc.scalar.copy(q_sb[:], in_ps[:])
    sw = tmp_pool.tile([128, SQ], BF16, tag="rope_sw")
    nc.vector.tensor_copy(sw[0:64, :], q_sb[64:128, :])
    nc.vector.tensor_copy(sw[64:128, :], q_sb[0:64, :])
    nc.vector.tensor_mul(sw[:], sw[:], sin_sl)
    nc.vector.tensor_mul(q_sb[:], q_sb[:], cos_sl)
    nc.vector.tensor_add(out_ap, q_sb[:], sw[:])


def build_kernel():
    nc = bass.Bass("TRN2", target_bir_lowering=False, debug=False,
                   num_devices=N_CORES)

    # All inputs are pre-tiled on the host into the exact sbuf layouts so
    # every DMA is contiguous per partition (few large descriptors).
    din = lambda n, shp, dt: nc.dram_tensor(n, shp, dt, kind="ExternalInput").ap()
    hsT_t = din("hsT_t", [NSL, 128, NK, SQ], BF16)
    NKQ = NK // 4
    wq_t = din("wq_t", [HQ, 128, NK, D], BF16)
    wk_t = din("wk_t", [128, NK, HKV * D], BF16)
    wv_t = din("wv_t", [128, NK, HKV * D], BF16)
    wo_t = din("wo_t", [DM // 512, 128, HQ, 512], BF16)
    wsq = din("wsq", [128, HQ, D], BF16)
    wsk = din("wsk", [128, HQ, D], BF16)
    cosT = din("cosT", [D, S], BF16)
    sinsg = din("sinsg", [D, S], BF16)
    maskbig = din("maskbig", [D, 896], BF16)
    out = nc.dram_tensor("out", [S, DM], BF16, kind="ExternalOutput").ap()

    with SplitDrainTileContext(nc) as tc, ExitStack() as octx:
        # ---------------- persistent sbuf ----------------
        pers = octx.enter_context(tc.tile_pool(name="pers", bufs=1))
        cos_sb = pers.tile([128, S], BF16, tag="cos")             # 4KB/p
        sin_sb = pers.tile([128, S], BF16, tag="sin")             # 4KB/p
        mask_sb = pers.tile([128, 896], BF16, tag="mask")
        wsq_sb = pers.tile([128, HQ, D], BF16, tag="wsq")         # 2KB/p
        wsk_sb = pers.tile([128, HQ, D], BF16, tag="wsk")
        ones128 = pers.tile([128, 128], BF16, tag="ones128")
        wk_sb = pers.tile([128, NK, HKV * D], BF16, tag="wk")     # 16KB/p
        wv_sb = pers.tile([128, NK, HKV * D], BF16, tag="wv")
        # per-slice K^T / V stay resident for the whole kernel (8+8KB/p);
        # separate tiles per slice so slice-n writes never conflict with
        # reads of earlier slices by in-flight attention
        kT_sl = [pers.tile([128, HKV, SQ], BF16, tag=f"kT{n}", name=f"kT{n}")
                 for n in range(NSL)]
        v_sl = [pers.tile([128, SQ // 128, HKV * D], BF16, tag=f"v{n}",
                          name=f"v{n}")
                for n in range(NSL)]

        hst_pool = octx.enter_context(tc.tile_pool(name="hstp", bufs=3))
        wq_pool = octx.enter_context(tc.tile_pool(name="wqp", bufs=2))
        wo_pool = octx.enter_context(tc.tile_pool(name="wop", bufs=2))
        qT_pool = octx.enter_context(tc.tile_pool(name="qTp", bufs=2))
        att_pool = octx.enter_context(tc.tile_pool(name="attp", bufs=3))
        ex_pool = octx.enter_context(tc.tile_pool(name="expp", bufs=6))
        exm_pool = octx.enter_context(tc.tile_pool(name="exmp", bufs=3))
        acc_pool = octx.enter_context(tc.tile_pool(name="accp", bufs=2))
        rtmp = octx.enter_context(tc.tile_pool(name="rtmp", bufs=2))
        fin_pool = octx.enter_context(tc.tile_pool(name="finp", bufs=2))
        od_pool = octx.enter_context(tc.tile_pool(name="odp", bufs=6))

        # PSUM bank budget (8): p_q 2 + p_s 2 + p_at 1 + p_dl 1 + p_o 2
        p_q = octx.enter_context(tc.tile_pool(name="p_q", bufs=2, space="PSUM"))
        p_s = octx.enter_context(tc.tile_pool(name="p_s", bufs=2, space="PSUM"))
        p_at = octx.enter_context(tc.tile_pool(name="p_at", bufs=1, space="PSUM"))
        p_dl = octx.enter_context(tc.tile_pool(name="p_dl", bufs=1, space="PSUM"))
        p_o = octx.enter_context(tc.tile_pool(name="p_o", bufs=2, space="PSUM"))

        # ---------------- startup loads ----------------
        # ordered so the first kv-proj chain can start ~6us in: first kv
        # head's weights + first half of hst(0), then the rest
        NKH = NK // 2

        def load_hst(n):
            """Two half-of-k tiles per slice, each DMA'd as two quarter
            chunks so the first matmuls unblock at quarter granularity."""
            hs = []
            for hh in range(2):
                ht = hst_pool.tile([128, NKH, SQ], BF16, tag="hst",
                                   name=f"hst{n}_{hh}")
                k0 = hh * NKH
                nc.sync.dma_start(ht[:, 0:NKQ, :], hsT_t[n, :, k0:k0 + NKQ, :])
                nc.sync.dma_start(ht[:, NKQ:NKH, :],
                                  hsT_t[n, :, k0 + NKQ:k0 + NKH, :])
                hs.append(ht)
            return hs

        nc.sync.dma_start(wk_sb[:, 0:NKH, 0:D], wk_t[:, 0:NKH, 0:D])
        nc.sync.dma_start(wk_sb[:, NKH:NK, 0:D], wk_t[:, NKH:NK, 0:D])
        hst0 = load_hst(0)
        nc.sync.dma_start(cos_sb[:], cosT[:, :])
        nc.sync.dma_start(sin_sb[:], sinsg[:, :])
        nc.sync.dma_start(wk_sb[:, :, D:HKV * D], wk_t[:, :, D:HKV * D])
        nc.sync.dma_start(wv_sb[:], wv_t[:, :, :])
        nc.any.memset(ones128[:], 1.0)
        nc.sync.dma_start(mask_sb[:], maskbig[:, :])
        nc.sync.dma_start(wsq_sb[:], wsq[:, :, :])
        nc.sync.dma_start(wsk_sb[:], wsk[:, :, :])

        att_tiles = {}

        # The final schedule is STATIC (the Tile list scheduler fixes each
        # engine's instruction order at compile time from its cost model,
        # and the model under-estimates the attention exp stalls). So the
        # o-projection is emitted as paced fill ITEMS woven into the
        # attention loops in program order: each item is ~2 matmuls, one
        # item per attention block keeps PE busy through the exp waits.
        class OFill:
            ITEMS_PER_PASS = (DM // 512) * (1 + 4 * 4)

            def __init__(self):
                self.gens = []
                self.count = 0

            def o_items(self, ns):
                att = att_tiles[ns]
                for j in range(DM // 512):
                    wo_sb = wo_pool.tile([128, HQ, 512], BF16, tag="wo",
                                         name=f"wo{ns}_{j}")
                    nc.sync.dma_start(wo_sb[:], wo_t[j])
                    yield
                    for m in range(4):
                        mg = ns * 4 + m
                        ps = p_o.tile([128, 512], F32, tag="ps_o",
                                      name=f"pso{ns}_{j}_{m}")
                        for t2 in range(HQ):
                            nc.tensor.matmul(
                                ps[:], att[:, t2, m * 128:(m + 1) * 128],
                                wo_sb[:, t2, :],
                                start=(t2 == 0), stop=(t2 == HQ - 1))
                            if t2 in (1, 3, 5):
                                yield
                        ot = od_pool.tile([128, 512], BF16, tag="ot",
                                          name=f"ot{ns}_{j}_{m}")
                        nc.vector.tensor_copy(ot[:], ps[:])
                        nc.sync.dma_start(out[mg * 128:(mg + 1) * 128,
                                              j * 512:(j + 1) * 512], ot[:])
                        yield

            def add(self, ns):
                self.gens.append(self.o_items(ns))
                self.count += self.ITEMS_PER_PASS

            def drain(self, k):
                while k > 0 and self.gens:
                    try:
                        next(self.gens[0])
                        self.count -= 1
                        k -= 1
                    except StopIteration:
                        self.gens.pop(0)

            def drain_all(self):
                self.drain(1 << 30)

        ofill = OFill()

        for n in range(NSL):
            sl = slice(n * SQ, (n + 1) * SQ)
            hst = hst0 if n == 0 else load_hst(n)
            hk = lambda k: hst[k // NKH][:, k % NKH, :]
            hk4 = lambda k, s4: hst[k // NKH][:, k % NKH, s4 * 128:(s4 + 1) * 128]

            # ---- k/v projections for this slice ----
            for kv in range(HKV):
                ps = p_q.tile([128, SQ], F32, tag="ps_p")
                for k in range(NK):
                    nc.tensor.matmul(
                        ps[:], wk_sb[:, k, kv * D:(kv + 1) * D], hk(k),
                        start=(k == 0), stop=(k == NK - 1))
                _rope(nc, rtmp, kT_sl[n][:, kv, :], ps,
                      cos_sb[:, sl], sin_sb[:, sl])
                ofill.drain(1)
            for s4 in range(SQ // 128):
                ps = p_q.tile([128, SQ], F32, tag="ps_p")
                for k in range(NK):
                    nc.tensor.matmul(
                        ps[:, 0:HKV * D], hk4(k, s4), wv_sb[:, k, :],
                        start=(k == 0), stop=(k == NK - 1))
                nc.scalar.copy(v_sl[n][:, s4, :], ps[:, 0:HKV * D])
                ofill.drain(1)

            # ---- q projection for this slice ----
            qT = qT_pool.tile([128, HQ, SQ], BF16, tag="qT")
            for h in range(HQ):
                wqc = wq_pool.tile([128, NK, D], BF16, tag="wqc")
                nc.sync.dma_start(wqc[:], wq_t[h])
                ps = p_q.tile([128, SQ], F32, tag="ps_p")
                for k in range(NK):
                    nc.tensor.matmul(ps[:], wqc[:, k, :], hk(k),
                                     start=(k == 0), stop=(k == NK - 1))
                _rope(nc, rtmp, qT[:, h, :], ps, cos_sb[:, sl], sin_sb[:, sl])
                ofill.drain(1)

            # ---- causal attention for this slice ----
            att = att_pool.tile([128, HQ, SQ], BF16, tag="att")
            att_tiles[n] = att
            nblk = 4 * (n + 1)
            # paced o-proj fill: spread the pending items evenly over this
            # attention's blocks so PE has ready work through the exp waits
            blocks_total = HQ * nblk
            fill0 = ofill.count
            blocks_done = 0
            filled = 0
            for h in range(HQ):
                kv = h // GROUPS
                qh_r = qT[:, h, :]
                ps_at = p_at.tile([128, SQ], F32, tag="ps_at")
                acc = acc_pool.tile([128, SQ], BF16, tag="acc")
                ex0 = None
                for t in range(nblk):
                    ps_sc = p_s.tile([128, SQ], F32, tag="ps_sc")
                    nc.tensor.matmul(
                        ps_sc[:],
                        kT_sl[t // 4][:, kv, (t % 4) * 128:(t % 4 + 1) * 128],
                        qh_r, start=True, stop=True)
                    ex = ex_pool.tile([128, SQ], BF16, tag="ex")
                    nc.scalar.activation(ex[:], ps_sc[:],
                                         mybir.ActivationFunctionType.Exp,
                                         scale=SCALE)
                    if t >= 4 * n:
                        r = t - 4 * n
                        exm = exm_pool.tile([128, SQ], BF16, tag="exm")
                        nc.vector.tensor_mul(
                            exm[:], ex[:], mask_sb[:, 384 - 128 * r: 896 - 128 * r])
                        ex = exm
                    nc.tensor.matmul(ps_at[:],
                                     v_sl[t // 4][:, t % 4, kv * D:(kv + 1) * D],
                                     ex[:], start=(t == 0), stop=(t == nblk - 1))
                    blocks_done += 1
                    want = fill0 * blocks_done // blocks_total
                    if want > filled:
                        ofill.drain(want - filled)
                        filled = want
                    # exp-sum accumulation on DVE (PE only does the final
                    # broadcast-reduce matmul below)
                    if t == 0:
                        ex0 = ex
                    elif t == 1:
                        nc.vector.tensor_add(acc[:], ex0[:], ex[:])
                    else:
                        nc.vector.tensor_add(acc[:], acc[:], ex[:])
                # sums broadcast to every partition: ones[128x128].T @ acc
                ps_sm = p_s.tile([128, SQ], F32, tag="ps_sc")
                nc.tensor.matmul(ps_sm[:], ones128[:], acc[:],
                                 start=True, stop=True)
                # 1/sums as exp(-ln(sums)) on ACT (DVE reciprocal is slow);
                # the Ln runs in place in the PSUM bank
                nc.scalar.activation(ps_sm[:], ps_sm[:],
                                     mybir.ActivationFunctionType.Ln)
                rc = fin_pool.tile([128, SQ], mybir.dt.float16, tag="rc")
                nc.scalar.activation(rc[:], ps_sm[:],
                                     mybir.ActivationFunctionType.Exp,
                                     scale=-1.0)
                # per-head shift correction
                ps_dl = p_dl.tile([128, SQ], F32, tag="ps_dl")
                nc.tensor.matmul(ps_dl[:], wsq_sb[:, h, :], qh_r,
                                 start=True, stop=False)
                nc.tensor.matmul(ps_dl[:], wsk_sb[:, h, :], kT_sl[n][:, kv, :],
                                 start=False, stop=True)
                t1 = fin_pool.tile([128, SQ], BF16, tag="t1")
                nc.vector.tensor_mul(t1[:], ps_at[:], rc[:])
                nc.vector.tensor_add(att[:, h, :], t1[:], ps_dl[:])

            ofill.add(n)
        ofill.drain_all()
    _split_excess_waits(nc)
    return nc


# ---------------------------------------------------------------- host side
_CACHE = {}


def _prep_core_inputs(inputs, core):
    b, g = core // TP, core % TP
    hs = np.asarray(inputs["hidden_states"])[b]          # [S, DM] f32
    pos = np.asarray(inputs["position_ids"])[b]          # [S] int32
    Wq, Wk, Wv, Wo = (np.asarray(inputs[k]) for k in ("Wq", "Wk", "Wv", "Wo"))
    Ws_q, Ws_k = np.asarray(inputs["Ws_q"]), np.asarray(inputs["Ws_k"])

    qh0 = g * HQ                 # first global q head
    kv0 = g * HKV                # first global kv head

    inv_freq = 1.0 / (THETA ** (np.arange(0, D, 2, dtype=np.float64) / D))
    freqs = pos.astype(np.float64)[:, None] * inv_freq[None, :]   # [S, 64]
    cos = np.cos(freqs).astype(np.float32)
    sin = np.sin(freqs).astype(np.float32)
    cosT = np.ascontiguousarray(np.concatenate([cos, cos], axis=1).T)   # [128,S]
    sinsg = np.ascontiguousarray(np.concatenate([-sin, sin], axis=1).T)

    ii = np.arange(128)[:, None]
    cc = np.arange(896)[None, :]
    maskbig = ((cc - 384) >= ii).astype(BF_NP)

    # pre-tile into exact on-chip layouts (contiguous per-partition DMAs)
    hsT = hs.T.astype(BF_NP)                                   # [DM, S]
    hsT_t = np.ascontiguousarray(
        hsT.reshape(NK, 128, NSL, SQ).transpose(2, 1, 0, 3))   # [n, p, k, s]
    wq_c = Wq[:, qh0 * D:(qh0 + HQ) * D].astype(BF_NP)         # [DM, 1024]
    wq_t = np.ascontiguousarray(
        wq_c.reshape(NK, 128, HQ, D).transpose(2, 1, 0, 3))    # [h, p, k, m]
    wk_c = Wk[:, kv0 * D:(kv0 + HKV) * D].astype(BF_NP)
    wk_t = np.ascontiguousarray(
        wk_c.reshape(NK, 128, HKV * D).transpose(1, 0, 2))     # [p, k, m]
    wv_c = Wv[:, kv0 * D:(kv0 + HKV) * D].astype(BF_NP)
    wv_t = np.ascontiguousarray(
        wv_c.reshape(NK, 128, HKV * D).transpose(1, 0, 2))
    wo_c = Wo[qh0 * D:(qh0 + HQ) * D, :].astype(BF_NP)         # [1024, DM]
    wo_t = np.ascontiguousarray(
        wo_c.reshape(HQ, 128, DM // 512, 512).transpose(2, 1, 0, 3))  # [j,p,t,m]
    wsq_t = np.ascontiguousarray(
        Ws_q[qh0:qh0 + HQ].transpose(1, 0, 2)).astype(np.float32)  # [d, h, e]
    wsk_t = np.ascontiguousarray(
        Ws_k[qh0:qh0 + HQ].transpose(1, 0, 2)).astype(np.float32)
    return {
        "hsT_t": hsT_t,
        "wq_t": wq_t,
        "wk_t": wk_t,
        "wv_t": wv_t,
        "wo_t": wo_t,
        "wsq": wsq_t.astype(BF_NP),
        "wsk": wsk_t.astype(BF_NP),
        "cosT": cosT.astype(BF_NP),
        "sinsg": sinsg.astype(BF_NP),
        "maskbig": maskbig,
    }


def run(inputs, trace=False):
    if "nc" not in _CACHE:
        _CACHE["nc"] = build_kernel()
    nc = _CACHE["nc"]
    in_maps = [_prep_core_inputs(inputs, c) for c in range(N_CORES)]
    res = bass_utils.run_bass_kernel_spmd(
        nc, in_maps, core_ids=list(range(N_CORES)), trace=trace)
    full = np.zeros((B, S, DM), dtype=np.float32)
    for c in range(N_CORES):
        full[c // TP] += np.asarray(res.results[c]["out"], dtype=np.float32)
    return full, res


def kernel(**inputs) -> np.ndarray:
    full, _ = run(inputs, trace=False)
    return full
